# revision 2
# baseline (speedup 1.0000x reference)
"""Grouped-expert FFN (MoE) kernel for Trainium2, expert-parallel over 8 NeuronCores.

Problem: x[16,2048,1024] @ w1[16,1024,4096] + b1 -> gelu -> @ w2[16,4096,1024] + b2.

Sharding: expert dim E=16 split as 2 experts per core (x, w1, w2 on axis 0;
b1/b2 replicated). Fully local grouped GEMM per core.

Device layout trick: host feeds xT[e] = x[e].T so that both GEMMs contract
along the SBUF partition dim with weights in their natural layout:
  GEMM1: hiddenT[h,n] = w1[d,h].T @ xT[d,n]   (lhsT = w1 tile, rhs = xT)
  GEMM2: outT[d,n]    = w2[h,d].T @ hiddenT[h,n]
Output is transposed back on the host.

All matmuls run in float32r (fp32 with 11-bit mantissa, full PE rate at
N=512 moving dim); PSUM accumulation is fp32; bias+GELU via ScalarE.
"""

import numpy as np

E_FULL = 16
N_TOK = 2048
D_DIM = 1024
H_DIM = 4096
N_CORES = 8
E_LOC = E_FULL // N_CORES  # 2 experts per core
NT = 1024                  # token half processed per phase (fits SBUF)
NB = 512                   # matmul moving-dim chunk (= one PSUM bank of fp32)

_CACHE = {}


def _round_fp32r(x: np.ndarray) -> np.ndarray:
    """Round fp32 -> fp32r (8-bit exp, 11-bit mantissa), RNE, in fp32 storage."""
    u = np.ascontiguousarray(x, dtype=np.float32).view(np.uint32)
    r = (u + np.uint32(0x7FF) + ((u >> np.uint32(12)) & np.uint32(1))) & np.uint32(
        0xFFFFF000
    )
    return r.view(np.float32)


def _build():
    from concourse import bass, tile, mybir, bacc

    F32R = mybir.dt.float32r
    F32 = mybir.dt.float32
    AF = mybir.ActivationFunctionType

    nc = bacc.Bacc("TRN2", target_bir_lowering=False, debug=False)

    xT = nc.dram_tensor("xT", (E_LOC, D_DIM, N_TOK), F32R, kind="ExternalInput").ap()
    w1 = nc.dram_tensor("w1", (E_LOC, D_DIM, H_DIM), F32R, kind="ExternalInput").ap()
    w2 = nc.dram_tensor("w2", (E_LOC, H_DIM, D_DIM), F32R, kind="ExternalInput").ap()
    b1c = nc.dram_tensor("b1c", (128, H_DIM // 128), F32, kind="ExternalInput").ap()
    b2c = nc.dram_tensor("b2c", (128, D_DIM // 128), F32, kind="ExternalInput").ap()
    outT = nc.dram_tensor("outT", (E_LOC, D_DIM, N_TOK), F32, kind="ExternalOutput").ap()

    KD = D_DIM // 128   # 8  k-tiles for GEMM1
    KH = H_DIM // 128   # 32 k-tiles for GEMM2
    MH = H_DIM // 128   # 32 m-tiles (hidden rows) for GEMM1
    MD = D_DIM // 128   # 8  m-tiles (out rows) for GEMM2
    NBS = NT // NB      # 2 moving chunks per phase

    with tile.TileContext(nc) as tc:
        with (
            tc.tile_pool(name="xp", bufs=9) as xp,
            tc.tile_pool(name="hp", bufs=MH + 1) as hp,
            tc.tile_pool(name="w1p", bufs=16) as w1p,
            tc.tile_pool(name="w2p", bufs=16) as w2p,
            tc.tile_pool(name="op", bufs=2) as op,
            tc.tile_pool(name="bp", bufs=1) as bp,
            tc.tile_pool(name="psa", bufs=4, space=bass.MemorySpace.PSUM) as psa,
            tc.tile_pool(name="psb", bufs=4, space=bass.MemorySpace.PSUM) as psb,
        ):
            b1t = bp.tile([128, H_DIM // 128], F32, tag="b1")
            b2t = bp.tile([128, D_DIM // 128], F32, tag="b2")
            nc.sync.dma_start(b1t[:], b1c[:])
            nc.sync.dma_start(b2t[:], b2c[:])

            for e in range(E_LOC):
                for t in range(N_TOK // NT):
                    # ---- load xT half: 8 tiles [128d, NT] ----
                    xts = []
                    for k in range(KD):
                        xt = xp.tile([128, NT], F32R, tag="x")
                        nc.sync.dma_start(
                            xt[:],
                            xT[e, k * 128 : (k + 1) * 128, t * NT : (t + 1) * NT],
                        )
                        xts.append(xt)

                    # ---- phase A: hiddenT[h, nt] = gelu(w1.T @ xT + b1) ----
                    hts = []
                    for m in range(MH):
                        pa = [psa.tile([128, NB], F32, tag="pa", name=f"pa{_}") for _ in range(NBS)]
                        for k in range(KD):
                            wt = w1p.tile([128, 128], F32R, tag="w1")
                            nc.sync.dma_start(
                                wt[:],
                                w1[e, k * 128 : (k + 1) * 128, m * 128 : (m + 1) * 128],
                            )
                            for nb in range(NBS):
                                nc.tensor.matmul(
                                    pa[nb][:],
                                    wt[:],
                                    xts[k][:, nb * NB : (nb + 1) * NB],
                                    start=(k == 0),
                                    stop=(k == KD - 1),
                                )
                        ht = hp.tile([128, NT], F32R, tag="h")
                        for nb in range(NBS):
                            nc.scalar.activation(
                                ht[:, nb * NB : (nb + 1) * NB],
                                pa[nb][:],
                                AF.Gelu,
                                bias=b1t[:, m : m + 1],
                            )
                        hts.append(ht)

                    # ---- phase B: outT[d, nt] = w2.T @ hiddenT + b2 ----
                    for m2 in range(MD):
                        pb = [psb.tile([128, NB], F32, tag="pb", name=f"pb{_}") for _ in range(NBS)]
                        for k in range(KH):
                            wt2 = w2p.tile([128, 128], F32R, tag="w2")
                            nc.sync.dma_start(
                                wt2[:],
                                w2[e, k * 128 : (k + 1) * 128, m2 * 128 : (m2 + 1) * 128],
                            )
                            for nb in range(NBS):
                                nc.tensor.matmul(
                                    pb[nb][:],
                                    wt2[:],
                                    hts[k][:, nb * NB : (nb + 1) * NB],
                                    start=(k == 0),
                                    stop=(k == KH - 1),
                                )
                        ot = op.tile([128, NT], F32, tag="o")
                        for nb in range(NBS):
                            nc.scalar.activation(
                                ot[:, nb * NB : (nb + 1) * NB],
                                pb[nb][:],
                                AF.Identity,
                                bias=b2t[:, m2 : m2 + 1],
                            )
                        nc.sync.dma_start(
                            outT[e, m2 * 128 : (m2 + 1) * 128, t * NT : (t + 1) * NT],
                            ot[:],
                        )

    nc.compile()
    return nc


def get_nc():
    if "nc" not in _CACHE:
        _CACHE["nc"] = _build()
    return _CACHE["nc"]


def make_in_maps(x, w1, w2, b1, b2):
    b1c = np.ascontiguousarray(b1.reshape(H_DIM // 128, 128).T, dtype=np.float32)
    b2c = np.ascontiguousarray(b2.reshape(D_DIM // 128, 128).T, dtype=np.float32)
    in_maps = []
    for c in range(N_CORES):
        sl = slice(E_LOC * c, E_LOC * (c + 1))
        in_maps.append(
            {
                "xT": _round_fp32r(x[sl].transpose(0, 2, 1)),
                "w1": _round_fp32r(w1[sl]),
                "w2": _round_fp32r(w2[sl]),
                "b1c": b1c,
                "b2c": b2c,
            }
        )
    return in_maps


def kernel(x, w1, w2, b1, b2):
    from concourse import bass_utils

    nc = get_nc()
    in_maps = make_in_maps(x, w1, w2, b1, b2)
    res = bass_utils.run_bass_kernel_spmd(nc, in_maps, core_ids=list(range(N_CORES)))
    out = np.empty((E_FULL, N_TOK, D_DIM), dtype=np.float32)
    for c in range(N_CORES):
        out[E_LOC * c : E_LOC * (c + 1)] = res.results[c]["outT"].transpose(0, 2, 1)
    return out


# revision 4
# speedup vs baseline: 8.2648x; 8.2648x over previous
"""Grouped-expert FFN (MoE) kernel for Trainium2, expert-parallel over 8 NeuronCores.

Problem: x[16,2048,1024] @ w1[16,1024,4096] + b1 -> gelu -> @ w2[16,4096,1024] + b2.

Sharding: expert dim E=16 split as 2 experts per core (x, w1, w2 on axis 0;
b1/b2 replicated). Fully local grouped GEMM per core.

Device layout trick: host feeds xT[e] = x[e].T so that both GEMMs contract
along the SBUF partition dim with weights in their natural layout:
  GEMM1: hiddenT[h,n] = w1[d,h].T @ xT[d,n]   (lhsT = w1 tile, rhs = xT)
  GEMM2: outT[d,n]    = w2[h,d].T @ hiddenT[h,n]
Output is transposed back on the host.

All matmuls run in float32r (fp32 with 11-bit mantissa, full PE rate at
N=512 moving dim); PSUM accumulation is fp32; bias+GELU via ScalarE.
"""

import numpy as np

E_FULL = 16
N_TOK = 2048
D_DIM = 1024
H_DIM = 4096
N_CORES = 8
E_LOC = E_FULL // N_CORES  # 2 experts per core
NT = 1024                  # token half processed per phase (fits SBUF)
NB = 512                   # matmul moving-dim chunk (= one PSUM bank of fp32)

_CACHE = {}


def _round_fp32r(x: np.ndarray) -> np.ndarray:
    """Round fp32 -> fp32r (8-bit exp, 11-bit mantissa), RNE, in fp32 storage."""
    u = np.ascontiguousarray(x, dtype=np.float32).view(np.uint32)
    r = (u + np.uint32(0x7FF) + ((u >> np.uint32(12)) & np.uint32(1))) & np.uint32(
        0xFFFFF000
    )
    return r.view(np.float32)


def _build(bench_iters=None):
    from concourse import bass, tile, mybir, bacc
    from contextlib import nullcontext

    F32R = mybir.dt.float32r
    F32 = mybir.dt.float32
    AF = mybir.ActivationFunctionType

    nc = bacc.Bacc("TRN2", target_bir_lowering=False, debug=False)

    xT = nc.dram_tensor("xT", (E_LOC, D_DIM, N_TOK), F32R, kind="ExternalInput").ap()
    w1 = nc.dram_tensor("w1", (E_LOC, D_DIM, H_DIM), F32R, kind="ExternalInput").ap()
    w2 = nc.dram_tensor("w2", (E_LOC, H_DIM, D_DIM), F32R, kind="ExternalInput").ap()
    b1c = nc.dram_tensor("b1c", (128, H_DIM // 128), F32, kind="ExternalInput").ap()
    b2c = nc.dram_tensor("b2c", (128, D_DIM // 128), F32, kind="ExternalInput").ap()
    outT = nc.dram_tensor("outT", (E_LOC, D_DIM, N_TOK), F32, kind="ExternalOutput").ap()

    KD = D_DIM // 128   # 8  k-tiles for GEMM1
    KH = H_DIM // 128   # 32 k-tiles for GEMM2
    MH = H_DIM // 128   # 32 m-tiles (hidden rows) for GEMM1
    MD = D_DIM // 128   # 8  m-tiles (out rows) for GEMM2
    NBS = NT // NB      # 2 moving chunks per phase

    with tile.TileContext(nc) as tc:
        with (
            tc.tile_pool(name="xp", bufs=9) as xp,
            tc.tile_pool(name="hp", bufs=MH + 1) as hp,
            tc.tile_pool(name="w1p", bufs=16) as w1p,
            tc.tile_pool(name="w2p", bufs=16) as w2p,
            tc.tile_pool(name="op", bufs=2) as op,
            tc.tile_pool(name="bp", bufs=1) as bp,
            tc.tile_pool(name="psa", bufs=4, space=bass.MemorySpace.PSUM) as psa,
            tc.tile_pool(name="psb", bufs=4, space=bass.MemorySpace.PSUM) as psb,
        ):
            loop_cm = (
                tc.For_i(
                    0,
                    bench_iters,
                    1,
                    hint_engines=(
                        mybir.EngineType.PE,
                        mybir.EngineType.Activation,
                        mybir.EngineType.SP,
                        mybir.EngineType.DVE,
                        mybir.EngineType.Pool,
                    ),
                )
                if bench_iters is not None
                else nullcontext()
            )
            with loop_cm:
              b1t = bp.tile([128, H_DIM // 128], F32, tag="b1")
              b2t = bp.tile([128, D_DIM // 128], F32, tag="b2")
              nc.sync.dma_start(b1t[:], b1c[:])
              nc.sync.dma_start(b2t[:], b2c[:])

              for e in range(E_LOC):
                for t in range(N_TOK // NT):
                    # ---- load xT half: 8 tiles [128d, NT] ----
                    xts = []
                    for k in range(KD):
                        xt = xp.tile([128, NT], F32R, tag="x")
                        nc.sync.dma_start(
                            xt[:],
                            xT[e, k * 128 : (k + 1) * 128, t * NT : (t + 1) * NT],
                        )
                        xts.append(xt)

                    # ---- phase A: hiddenT[h, nt] = gelu(w1.T @ xT + b1) ----
                    hts = []
                    for m in range(MH):
                        pa = [psa.tile([128, NB], F32, tag="pa", name=f"pa{_}") for _ in range(NBS)]
                        for k in range(KD):
                            wt = w1p.tile([128, 128], F32R, tag="w1")
                            nc.sync.dma_start(
                                wt[:],
                                w1[e, k * 128 : (k + 1) * 128, m * 128 : (m + 1) * 128],
                            )
                            for nb in range(NBS):
                                nc.tensor.matmul(
                                    pa[nb][:],
                                    wt[:],
                                    xts[k][:, nb * NB : (nb + 1) * NB],
                                    start=(k == 0),
                                    stop=(k == KD - 1),
                                )
                        ht = hp.tile([128, NT], F32R, tag="h")
                        for nb in range(NBS):
                            nc.scalar.activation(
                                ht[:, nb * NB : (nb + 1) * NB],
                                pa[nb][:],
                                AF.Gelu,
                                bias=b1t[:, m : m + 1],
                            )
                        hts.append(ht)

                    # ---- phase B: outT[d, nt] = w2.T @ hiddenT + b2 ----
                    for m2 in range(MD):
                        pb = [psb.tile([128, NB], F32, tag="pb", name=f"pb{_}") for _ in range(NBS)]
                        for k in range(KH):
                            wt2 = w2p.tile([128, 128], F32R, tag="w2")
                            nc.sync.dma_start(
                                wt2[:],
                                w2[e, k * 128 : (k + 1) * 128, m2 * 128 : (m2 + 1) * 128],
                            )
                            for nb in range(NBS):
                                nc.tensor.matmul(
                                    pb[nb][:],
                                    wt2[:],
                                    hts[k][:, nb * NB : (nb + 1) * NB],
                                    start=(k == 0),
                                    stop=(k == KH - 1),
                                )
                        ot = op.tile([128, NT], F32, tag="o")
                        for nb in range(NBS):
                            nc.scalar.activation(
                                ot[:, nb * NB : (nb + 1) * NB],
                                pb[nb][:],
                                AF.Identity,
                                bias=b2t[:, m2 : m2 + 1],
                            )
                        nc.sync.dma_start(
                            outT[e, m2 * 128 : (m2 + 1) * 128, t * NT : (t + 1) * NT],
                            ot[:],
                        )

    nc.compile()
    return nc


def get_nc():
    if "nc" not in _CACHE:
        _CACHE["nc"] = _build()
    return _CACHE["nc"]


def make_in_maps(x, w1, w2, b1, b2):
    b1c = np.ascontiguousarray(b1.reshape(H_DIM // 128, 128).T, dtype=np.float32)
    b2c = np.ascontiguousarray(b2.reshape(D_DIM // 128, 128).T, dtype=np.float32)
    in_maps = []
    for c in range(N_CORES):
        sl = slice(E_LOC * c, E_LOC * (c + 1))
        in_maps.append(
            {
                "xT": _round_fp32r(x[sl].transpose(0, 2, 1)),
                "w1": _round_fp32r(w1[sl]),
                "w2": _round_fp32r(w2[sl]),
                "b1c": b1c,
                "b2c": b2c,
            }
        )
    return in_maps


def kernel(x, w1, w2, b1, b2):
    from concourse import bass_utils

    nc = get_nc()
    in_maps = make_in_maps(x, w1, w2, b1, b2)
    res = bass_utils.run_bass_kernel_spmd(nc, in_maps, core_ids=list(range(N_CORES)))
    out = np.empty((E_FULL, N_TOK, D_DIM), dtype=np.float32)
    for c in range(N_CORES):
        out[E_LOC * c : E_LOC * (c + 1)] = res.results[c]["outT"].transpose(0, 2, 1)
    return out


# revision 8
# speedup vs baseline: 11.4351x; 1.3836x over previous
"""Grouped-expert FFN (MoE) kernel for Trainium2, expert-parallel over 8 NeuronCores.

Problem: x[16,2048,1024] @ w1[16,1024,4096] + b1 -> gelu -> @ w2[16,4096,1024] + b2.

Sharding: expert dim E=16 split as 2 experts per core (x, w1, w2 on axis 0;
b1/b2 replicated). Fully local grouped GEMM per core.

Device layout trick: host feeds xT[e] = x[e].T so that both GEMMs contract
along the SBUF partition dim with weights in their natural layout:
  GEMM1: hiddenT[h,n] = w1[d,h].T @ xT[d,n]   (lhsT = w1 tile, rhs = xT)
  GEMM2: outT[d,n]    = w2[h,d].T @ hiddenT[h,n]
Output is transposed back on the host.

All matmuls run in float32r (fp32 with 11-bit mantissa, full PE rate at
N=512 moving dim); PSUM accumulation is fp32; bias+GELU via ScalarE.
"""

import numpy as np

E_FULL = 16
N_TOK = 2048
D_DIM = 1024
H_DIM = 4096
N_CORES = 8
E_LOC = E_FULL // N_CORES  # 2 experts per core
NT = 1024                  # token half processed per phase (fits SBUF)
NB = 512                   # matmul moving-dim chunk (= one PSUM bank of fp32)

_CACHE = {}


def _round_fp32r(x: np.ndarray) -> np.ndarray:
    """Round fp32 -> fp32r (8-bit exp, 11-bit mantissa), RNE, in fp32 storage."""
    u = np.ascontiguousarray(x, dtype=np.float32).view(np.uint32)
    r = (u + np.uint32(0x7FF) + ((u >> np.uint32(12)) & np.uint32(1))) & np.uint32(
        0xFFFFF000
    )
    return r.view(np.float32)


def _build(bench_iters=None):
    from concourse import bass, tile, mybir, bacc
    from contextlib import nullcontext

    F32R = mybir.dt.float32r
    F32 = mybir.dt.float32
    AF = mybir.ActivationFunctionType

    nc = bacc.Bacc("TRN2", target_bir_lowering=False, debug=False)

    xT = nc.dram_tensor("xT", (E_LOC, D_DIM, N_TOK), F32R, kind="ExternalInput").ap()
    w1 = nc.dram_tensor("w1", (E_LOC, D_DIM, H_DIM), F32R, kind="ExternalInput").ap()
    w2 = nc.dram_tensor("w2", (E_LOC, H_DIM, D_DIM), F32R, kind="ExternalInput").ap()
    b1c = nc.dram_tensor("b1c", (128, H_DIM // 128), F32, kind="ExternalInput").ap()
    b2c = nc.dram_tensor("b2c", (128, D_DIM // 128), F32, kind="ExternalInput").ap()
    outT = nc.dram_tensor("outT", (E_LOC, D_DIM, N_TOK), F32, kind="ExternalOutput").ap()

    KD = D_DIM // 128   # 8  k-tiles for GEMM1
    KH = H_DIM // 128   # 32 k-tiles for GEMM2
    MH = H_DIM // 128   # 32 m-tiles (hidden rows) for GEMM1
    MD = D_DIM // 128   # 8  m-tiles (out rows) for GEMM2
    NBS = NT // NB      # 2 moving chunks per phase

    with tile.TileContext(nc) as tc:
        with (
            tc.tile_pool(name="xp", bufs=8) as xp,
            tc.tile_pool(name="hp", bufs=MH) as hp,
            tc.tile_pool(name="w1p", bufs=4) as w1p,
            tc.tile_pool(name="w2p", bufs=4) as w2p,
            tc.tile_pool(name="op", bufs=2) as op,
            tc.tile_pool(name="bp", bufs=1) as bp,
            tc.tile_pool(name="psa", bufs=4, space=bass.MemorySpace.PSUM) as psa,
            tc.tile_pool(name="psb", bufs=4, space=bass.MemorySpace.PSUM) as psb,
        ):
            loop_cm = (
                tc.For_i(
                    0,
                    bench_iters,
                    1,
                    hint_engines=(
                        mybir.EngineType.PE,
                        mybir.EngineType.Activation,
                        mybir.EngineType.SP,
                        mybir.EngineType.DVE,
                        mybir.EngineType.Pool,
                    ),
                )
                if bench_iters is not None
                else nullcontext()
            )
            with loop_cm:
              b1t = bp.tile([128, H_DIM // 128], F32, tag="b1")
              b2t = bp.tile([128, D_DIM // 128], F32, tag="b2")
              nc.sync.dma_start(b1t[:], b1c[:])
              nc.sync.dma_start(b2t[:], b2c[:])

              for e in range(E_LOC):
                for t in range(N_TOK // NT):
                    # ---- load xT half: 8 tiles [128d, NT] ----
                    xts = []
                    for k in range(KD):
                        xt = xp.tile([128, NT], F32R, tag="x")
                        nc.sync.dma_start(
                            xt[:],
                            xT[e, k * 128 : (k + 1) * 128, t * NT : (t + 1) * NT],
                        )
                        xts.append(xt)

                    # ---- phase A: hiddenT[h, nt] = gelu(w1.T @ xT + b1) ----
                    hts = []
                    for m in range(MH):
                        pa = [psa.tile([128, NB], F32, tag="pa", name=f"pa{_}") for _ in range(NBS)]
                        # one blocked DMA: all 8 k-tiles of w1 column-block m
                        wblk = w1p.tile([128, KD * 128], F32R, tag="w1", name="wblk")
                        nc.sync.dma_start(
                            wblk[:],
                            w1[e].rearrange("(kd p) h -> p kd h", p=128)[
                                :, :, m * 128 : (m + 1) * 128
                            ],
                        )
                        for k in range(KD):
                            for nb in range(NBS):
                                nc.tensor.matmul(
                                    pa[nb][:],
                                    wblk[:, k * 128 : (k + 1) * 128],
                                    xts[k][:, nb * NB : (nb + 1) * NB],
                                    start=(k == 0),
                                    stop=(k == KD - 1),
                                )
                        ht = hp.tile([128, NT], F32R, tag="h")
                        for nb in range(NBS):
                            nc.scalar.activation(
                                ht[:, nb * NB : (nb + 1) * NB],
                                pa[nb][:],
                                AF.Gelu,
                                bias=b1t[:, m : m + 1],
                            )
                        hts.append(ht)

                    # ---- phase B: outT[d, nt] = w2.T @ hiddenT + b2 ----
                    for m2 in range(MD):
                        pb = [psb.tile([128, NB], F32, tag="pb", name=f"pb{_}") for _ in range(NBS)]
                        for g in range(KH // 8):
                            # one blocked DMA: 8 k-tiles of w2 column-block m2
                            wblk2 = w2p.tile([128, 8 * 128], F32R, tag="w2", name="wblk2")
                            nc.sync.dma_start(
                                wblk2[:],
                                w2[e].rearrange("(kh p) d -> p kh d", p=128)[
                                    :,
                                    g * 8 : (g + 1) * 8,
                                    m2 * 128 : (m2 + 1) * 128,
                                ],
                            )
                            for ki in range(8):
                                k = g * 8 + ki
                                for nb in range(NBS):
                                    nc.tensor.matmul(
                                        pb[nb][:],
                                        wblk2[:, ki * 128 : (ki + 1) * 128],
                                        hts[k][:, nb * NB : (nb + 1) * NB],
                                        start=(k == 0),
                                        stop=(k == KH - 1),
                                    )
                        ot = op.tile([128, NT], F32, tag="o")
                        for nb in range(NBS):
                            nc.scalar.activation(
                                ot[:, nb * NB : (nb + 1) * NB],
                                pb[nb][:],
                                AF.Identity,
                                bias=b2t[:, m2 : m2 + 1],
                            )
                        nc.sync.dma_start(
                            outT[e, m2 * 128 : (m2 + 1) * 128, t * NT : (t + 1) * NT],
                            ot[:],
                        )

    nc.compile()
    return nc


def get_nc():
    if "nc" not in _CACHE:
        _CACHE["nc"] = _build()
    return _CACHE["nc"]


def make_in_maps(x, w1, w2, b1, b2):
    b1c = np.ascontiguousarray(b1.reshape(H_DIM // 128, 128).T, dtype=np.float32)
    b2c = np.ascontiguousarray(b2.reshape(D_DIM // 128, 128).T, dtype=np.float32)
    in_maps = []
    for c in range(N_CORES):
        sl = slice(E_LOC * c, E_LOC * (c + 1))
        in_maps.append(
            {
                "xT": _round_fp32r(x[sl].transpose(0, 2, 1)),
                "w1": _round_fp32r(w1[sl]),
                "w2": _round_fp32r(w2[sl]),
                "b1c": b1c,
                "b2c": b2c,
            }
        )
    return in_maps


def kernel(x, w1, w2, b1, b2):
    from concourse import bass_utils

    nc = get_nc()
    in_maps = make_in_maps(x, w1, w2, b1, b2)
    res = bass_utils.run_bass_kernel_spmd(nc, in_maps, core_ids=list(range(N_CORES)))
    out = np.empty((E_FULL, N_TOK, D_DIM), dtype=np.float32)
    for c in range(N_CORES):
        out[E_LOC * c : E_LOC * (c + 1)] = res.results[c]["outT"].transpose(0, 2, 1)
    return out


# revision 12
# speedup vs baseline: 12.0603x; 1.0547x over previous
"""Grouped-expert FFN (MoE) kernel for Trainium2, expert-parallel over 8 NeuronCores.

Problem: x[16,2048,1024] @ w1[16,1024,4096] + b1 -> gelu -> @ w2[16,4096,1024] + b2.

Sharding: expert dim E=16 split as 2 experts per core (x, w1, w2 on axis 0;
b1/b2 replicated). Fully local grouped GEMM per core.

Device layout trick: host feeds xT[e] = x[e].T so that both GEMMs contract
along the SBUF partition dim with weights in their natural layout:
  GEMM1: hiddenT[h,n] = w1[d,h].T @ xT[d,n]   (lhsT = w1 tile, rhs = xT)
  GEMM2: outT[d,n]    = w2[h,d].T @ hiddenT[h,n]
Output is transposed back on the host.

All matmuls run in float32r (fp32 with 11-bit mantissa, full PE rate at
N=512 moving dim); PSUM accumulation is fp32; bias+GELU via ScalarE.
"""

import numpy as np

E_FULL = 16
N_TOK = 2048
D_DIM = 1024
H_DIM = 4096
N_CORES = 8
E_LOC = E_FULL // N_CORES  # 2 experts per core
NT = 1024                  # token half processed per phase (fits SBUF)
NB = 512                   # matmul moving-dim chunk (= one PSUM bank of fp32)

_CACHE = {}


def _round_fp32r(x: np.ndarray) -> np.ndarray:
    """Round fp32 -> fp32r (8-bit exp, 11-bit mantissa), RNE, in fp32 storage."""
    u = np.ascontiguousarray(x, dtype=np.float32).view(np.uint32)
    r = (u + np.uint32(0x7FF) + ((u >> np.uint32(12)) & np.uint32(1))) & np.uint32(
        0xFFFFF000
    )
    return r.view(np.float32)


def _build(bench_iters=None):
    from concourse import bass, tile, mybir, bacc
    from contextlib import nullcontext

    F32R = mybir.dt.float32r
    F32 = mybir.dt.float32
    AF = mybir.ActivationFunctionType

    nc = bacc.Bacc("TRN2", target_bir_lowering=False, debug=False)

    KD_ = D_DIM // 128
    KH_ = H_DIM // 128
    xT = nc.dram_tensor("xT", (E_LOC, D_DIM, N_TOK), F32R, kind="ExternalInput").ap()
    # host-swizzled: w1s[e, m, p, k*128+j] = w1[e, k*128+p, m*128+j]
    w1 = nc.dram_tensor(
        "w1s", (E_LOC, KH_, 128, KD_ * 128), F32R, kind="ExternalInput"
    ).ap()
    # host-swizzled: w2s[e, m2, g, p, ki*128+j] = w2[e, (g*8+ki)*128+p, m2*128+j]
    w2 = nc.dram_tensor(
        "w2s", (E_LOC, KD_, KH_ // 8, 128, 8 * 128), F32R, kind="ExternalInput"
    ).ap()
    b1c = nc.dram_tensor("b1c", (128, H_DIM // 128), F32, kind="ExternalInput").ap()
    b2c = nc.dram_tensor("b2c", (128, D_DIM // 128), F32, kind="ExternalInput").ap()
    outT = nc.dram_tensor("outT", (E_LOC, D_DIM, N_TOK), F32, kind="ExternalOutput").ap()

    KD = D_DIM // 128   # 8  k-tiles for GEMM1
    KH = H_DIM // 128   # 32 k-tiles for GEMM2
    MH = H_DIM // 128   # 32 m-tiles (hidden rows) for GEMM1
    MD = D_DIM // 128   # 8  m-tiles (out rows) for GEMM2
    NBS = NT // NB      # 2 moving chunks per phase

    with tile.TileContext(nc) as tc:
        with (
            tc.tile_pool(name="xp", bufs=8) as xp,
            tc.tile_pool(name="hp", bufs=MH) as hp,
            tc.tile_pool(name="w1p", bufs=4) as w1p,
            tc.tile_pool(name="w2p", bufs=4) as w2p,
            tc.tile_pool(name="op", bufs=2) as op,
            tc.tile_pool(name="bp", bufs=1) as bp,
            tc.tile_pool(name="psa", bufs=4, space=bass.MemorySpace.PSUM) as psa,
            tc.tile_pool(name="psb", bufs=4, space=bass.MemorySpace.PSUM) as psb,
        ):
            loop_cm = (
                tc.For_i(
                    0,
                    bench_iters,
                    1,
                    hint_engines=(
                        mybir.EngineType.PE,
                        mybir.EngineType.Activation,
                        mybir.EngineType.SP,
                        mybir.EngineType.DVE,
                        mybir.EngineType.Pool,
                    ),
                )
                if bench_iters is not None
                else nullcontext()
            )
            with loop_cm:
              b1t = bp.tile([128, H_DIM // 128], F32, tag="b1")
              b2t = bp.tile([128, D_DIM // 128], F32, tag="b2")
              nc.sync.dma_start(b1t[:], b1c[:])
              nc.sync.dma_start(b2t[:], b2c[:])

              for e in range(E_LOC):
                for t in range(N_TOK // NT):
                    # ---- load xT half: 8 tiles [128d, NT] ----
                    xts = []
                    for k in range(KD):
                        xt = xp.tile([128, NT], F32R, tag="x")
                        nc.sync.dma_start(
                            xt[:],
                            xT[e, k * 128 : (k + 1) * 128, t * NT : (t + 1) * NT],
                        )
                        xts.append(xt)

                    # ---- phase A: hiddenT[h, nt] = gelu(w1.T @ xT + b1) ----
                    hts = []
                    for m in range(MH):
                        pa = [psa.tile([128, NB], F32, tag="pa", name=f"pa{_}") for _ in range(NBS)]
                        # one blocked DMA: all 8 k-tiles of w1 column-block m
                        wblk = w1p.tile([128, KD * 128], F32R, tag="w1", name="wblk")
                        nc.sync.dma_start(wblk[:], w1[e, m])
                        for k in range(KD):
                            for nb in range(NBS):
                                nc.tensor.matmul(
                                    pa[nb][:],
                                    wblk[:, k * 128 : (k + 1) * 128],
                                    xts[k][:, nb * NB : (nb + 1) * NB],
                                    start=(k == 0),
                                    stop=(k == KD - 1),
                                )
                        ht = hp.tile([128, NT], F32R, tag="h")
                        for nb in range(NBS):
                            nc.scalar.activation(
                                ht[:, nb * NB : (nb + 1) * NB],
                                pa[nb][:],
                                AF.Gelu,
                                bias=b1t[:, m : m + 1],
                            )
                        hts.append(ht)

                    # ---- phase B: outT[d, nt] = w2.T @ hiddenT + b2 ----
                    for m2 in range(MD):
                        pb = [psb.tile([128, NB], F32, tag="pb", name=f"pb{_}") for _ in range(NBS)]
                        for g in range(KH // 8):
                            # one blocked DMA: 8 k-tiles of w2 column-block m2
                            wblk2 = w2p.tile([128, 8 * 128], F32R, tag="w2", name="wblk2")
                            nc.sync.dma_start(wblk2[:], w2[e, m2, g])
                            for ki in range(8):
                                k = g * 8 + ki
                                for nb in range(NBS):
                                    nc.tensor.matmul(
                                        pb[nb][:],
                                        wblk2[:, ki * 128 : (ki + 1) * 128],
                                        hts[k][:, nb * NB : (nb + 1) * NB],
                                        start=(k == 0),
                                        stop=(k == KH - 1),
                                    )
                        ot = op.tile([128, NT], F32, tag="o")
                        for nb in range(NBS):
                            nc.scalar.activation(
                                ot[:, nb * NB : (nb + 1) * NB],
                                pb[nb][:],
                                AF.Identity,
                                bias=b2t[:, m2 : m2 + 1],
                            )
                        nc.sync.dma_start(
                            outT[e, m2 * 128 : (m2 + 1) * 128, t * NT : (t + 1) * NT],
                            ot[:],
                        )

    nc.compile()
    return nc


def get_nc():
    if "nc" not in _CACHE:
        _CACHE["nc"] = _build()
    return _CACHE["nc"]


def _swizzle_w1(w1_loc):
    # [E, D, H] -> [E, MH, 128p, KD*128] with w1s[e,m,p,k*128+j] = w1[e,k*128+p,m*128+j]
    e = w1_loc.shape[0]
    v = w1_loc.reshape(e, D_DIM // 128, 128, H_DIM // 128, 128)  # e,k,p,m,j
    return np.ascontiguousarray(v.transpose(0, 3, 2, 1, 4)).reshape(
        e, H_DIM // 128, 128, (D_DIM // 128) * 128
    )


def _swizzle_w2(w2_loc):
    # [E, H, D] -> [E, MD, G, 128p, 8*128] with w2s[e,m2,g,p,ki*128+j] = w2[e,(g*8+ki)*128+p,m2*128+j]
    e = w2_loc.shape[0]
    v = w2_loc.reshape(e, H_DIM // 1024, 8, 128, D_DIM // 128, 128)  # e,g,ki,p,m2,j
    return np.ascontiguousarray(v.transpose(0, 4, 1, 3, 2, 5)).reshape(
        e, D_DIM // 128, H_DIM // 1024, 128, 8 * 128
    )


def make_in_maps(x, w1, w2, b1, b2):
    b1c = np.ascontiguousarray(b1.reshape(H_DIM // 128, 128).T, dtype=np.float32)
    b2c = np.ascontiguousarray(b2.reshape(D_DIM // 128, 128).T, dtype=np.float32)
    in_maps = []
    for c in range(N_CORES):
        sl = slice(E_LOC * c, E_LOC * (c + 1))
        in_maps.append(
            {
                "xT": _round_fp32r(x[sl].transpose(0, 2, 1)),
                "w1s": _swizzle_w1(_round_fp32r(w1[sl])),
                "w2s": _swizzle_w2(_round_fp32r(w2[sl])),
                "b1c": b1c,
                "b2c": b2c,
            }
        )
    return in_maps


def kernel(x, w1, w2, b1, b2):
    from concourse import bass_utils

    nc = get_nc()
    in_maps = make_in_maps(x, w1, w2, b1, b2)
    res = bass_utils.run_bass_kernel_spmd(nc, in_maps, core_ids=list(range(N_CORES)))
    out = np.empty((E_FULL, N_TOK, D_DIM), dtype=np.float32)
    for c in range(N_CORES):
        out[E_LOC * c : E_LOC * (c + 1)] = res.results[c]["outT"].transpose(0, 2, 1)
    return out
